# revision 2
# baseline (speedup 1.0000x reference)
"""AttentionDTI forward pass on 8 TRN2 NeuronCores — pure data parallel over batch.

Model (B=8, LD=100, LP=1000, DIM=64, CONV=40, C4=160):
  embed -> 3x conv1d+relu (drug: k=4,6,8 ; protein: k=4,8,12)
  d_att = dc^T @ d_att_w + b ; p_att = pc^T @ p_att_w + b
  R = relu(d_att[:,i,None,:] + p_att[:,None,j,:])      # [B,85,979,160] never materialized in DRAM
  comp_atte = sigmoid((R.mean(2) @ att_w + att_b)^T)   # via S[c,i] = sum_j relu(...)
  prot_atte = sigmoid((R.mean(1) @ att_w + att_b)^T)   # via T[c,j] = sum_i relu(...)
  gate, global max pool, FC 320->1024->1024->512->2 (leaky relu 0.01)

Sharding: core b handles batch element b. All params replicated. No collectives.
R-loop per core: for each i, ScalarE computes tmp=relu(P + D[:,i]) with fused
free-axis accumulation (S column); TensorE accumulates T += tmp via identity
matmul into PSUM.
"""

import numpy as np

B, LD, LP, DIM, CONV = 8, 100, 1000, 64, 40
C4 = 160
LD1, LD2, LD3 = 97, 92, 85     # drug conv output lengths (k=4,6,8)
LP1, LP2, LP3 = 997, 990, 979  # protein conv output lengths (k=4,8,12)

_CACHE = {}


def _build(dt_name="float32"):
    from contextlib import ExitStack
    import concourse.bass as bass
    import concourse.tile as tile
    from concourse import bacc, mybir

    f32 = mybir.dt.float32
    AF = mybir.ActivationFunctionType
    ALU = mybir.AluOpType
    AX = mybir.AxisListType

    nc = bacc.Bacc("TRN2", target_bir_lowering=False, debug=False)

    def par(name, shape):
        return nc.declare_dram_parameter(name, list(shape), f32, isOutput=False)

    # per-core inputs
    d_idx = par("drug_idx", [1, LD])
    p_idx = par("prot_idx", [1, LP])
    # replicated params
    emb_d = par("drug_emb", [65, DIM])
    emb_p = par("prot_emb", [26, DIM])
    dw1t = par("dw1t", [4, DIM, CONV])
    dw2t = par("dw2t", [6, CONV, 2 * CONV])
    dw3t = par("dw3t", [8, 2 * CONV, C4])
    pw1t = par("pw1t", [4, DIM, CONV])
    pw2t = par("pw2t", [8, CONV, 2 * CONV])
    pw3t = par("pw3t", [12, 2 * CONV, C4])
    db1 = par("db1", [CONV, 1])
    db2 = par("db2", [2 * CONV, 1])
    db3 = par("db3", [2, 80, 1])
    pb1 = par("pb1", [CONV, 1])
    pb2 = par("pb2", [2 * CONV, 1])
    pb3 = par("pb3", [2, 80, 1])
    daw = par("d_att_w", [C4, C4])
    paw = par("p_att_w", [C4, C4])
    aw = par("att_w", [C4, C4])
    dab = par("d_att_b", [2, 80, 1])
    pab = par("p_att_b", [2, 80, 1])
    ab = par("att_b", [2, 80, 1])
    fc1w = par("fc1_w", [320, 1024])
    fc1b = par("fc1_b", [128, 8])
    fc2w = par("fc2_w", [1024, 1024])
    fc2b = par("fc2_b", [128, 8])
    fc3w = par("fc3_w", [1024, 512])
    fc3b = par("fc3_b", [128, 4])
    outw = par("out_w", [512, 2])
    outb = par("out_b", [2, 1])
    ones1 = par("ones1", [1, 128])
    iota = par("iota", [128, 1])
    ident = par("ident80", [80, 80])

    out_d = nc.declare_dram_parameter("out", [2, 1], f32, isOutput=True)

    with tile.TileContext(nc) as tc, ExitStack() as ctx:
        wp = ctx.enter_context(tc.tile_pool(name="w", bufs=1))
        ap_ = ctx.enter_context(tc.tile_pool(name="a", bufs=1))
        tp = ctx.enter_context(tc.tile_pool(name="t", bufs=4))
        pp = ctx.enter_context(tc.tile_pool(name="p", bufs=2, space="PSUM"))
        pT = ctx.enter_context(tc.tile_pool(name="pT", bufs=1, space="PSUM"))

        def load(dram_ap, shape, tag):
            t = wp.tile(list(shape), f32, tag=tag)
            nc.sync.dma_start(out=t[:], in_=dram_ap)
            return t

        # ---- load weights/constants ----
        ones_t = load(ones1[:], [1, 128], "ones")
        iota_t = load(iota[:], [128, 1], "iota")
        ident_t = load(ident[:], [80, 80], "ident")
        embd_t = load(emb_d[:], [65, DIM], "embd")
        embp_t = load(emb_p[:], [26, DIM], "embp")
        dw1_t = [load(dw1t[k], [DIM, CONV], f"dw1_{k}") for k in range(4)]
        dw2_t = [load(dw2t[k], [CONV, 2 * CONV], f"dw2_{k}") for k in range(6)]
        dw3_t = [load(dw3t[k], [2 * CONV, C4], f"dw3_{k}") for k in range(8)]
        pw1_t = [load(pw1t[k], [DIM, CONV], f"pw1_{k}") for k in range(4)]
        pw2_t = [load(pw2t[k], [CONV, 2 * CONV], f"pw2_{k}") for k in range(8)]
        pw3_t = [load(pw3t[k], [2 * CONV, C4], f"pw3_{k}") for k in range(12)]
        db1_t = load(db1[:], [CONV, 1], "db1")
        db2_t = load(db2[:], [2 * CONV, 1], "db2")
        db3_t = [load(db3[i], [80, 1], f"db3_{i}") for i in range(2)]
        pb1_t = load(pb1[:], [CONV, 1], "pb1")
        pb2_t = load(pb2[:], [2 * CONV, 1], "pb2")
        pb3_t = [load(pb3[i], [80, 1], f"pb3_{i}") for i in range(2)]
        daw_t = [load(daw[c * 80:(c + 1) * 80, :], [80, C4], f"daw_{c}") for c in range(2)]
        paw_t = [load(paw[c * 80:(c + 1) * 80, :], [80, C4], f"paw_{c}") for c in range(2)]
        aw_t = [load(aw[c * 80:(c + 1) * 80, :], [80, C4], f"aw_{c}") for c in range(2)]
        dab_t = [load(dab[i], [80, 1], f"dab_{i}") for i in range(2)]
        pab_t = [load(pab[i], [80, 1], f"pab_{i}") for i in range(2)]
        ab_t = [load(ab[i], [80, 1], f"ab_{i}") for i in range(2)]
        fc1w_t = [load(fc1w[g * 80:(g + 1) * 80, :], [80, 1024], f"fc1w_{g}") for g in range(4)]
        fc2w_t = [load(fc2w[g * 128:(g + 1) * 128, :], [128, 1024], f"fc2w_{g}") for g in range(8)]
        fc3w_t = [load(fc3w[g * 128:(g + 1) * 128, :], [128, 512], f"fc3w_{g}") for g in range(8)]
        outw_t = [load(outw[g * 128:(g + 1) * 128, :], [128, 2], f"outw_{g}") for g in range(4)]
        fc1b_t = load(fc1b[:], [128, 8], "fc1b")
        fc2b_t = load(fc2b[:], [128, 8], "fc2b")
        fc3b_t = load(fc3b[:], [128, 4], "fc3b")
        outb_t = load(outb[:], [2, 1], "outb")

        # ---- one-hot + embedding ----
        def embed(idx_dram, nvocab, L, emb_t, tag):
            idx_t = ap_.tile([1, L], f32, tag=f"idx_{tag}")
            nc.sync.dma_start(out=idx_t[:], in_=idx_dram)
            e = ap_.tile([DIM, L], f32, tag=f"e_{tag}")
            for l0 in range(0, L, 512):
                cs = min(512, L - l0)
                psb = pp.tile([nvocab, 512], f32, tag="ps")
                nc.tensor.matmul(psb[:, :cs], ones_t[:, :nvocab], idx_t[:, l0:l0 + cs],
                                 start=True, stop=True)
                oh = tp.tile([nvocab, 512], f32, tag="oh")
                nc.vector.tensor_scalar(out=oh[:, :cs], in0=psb[:, :cs],
                                        scalar1=iota_t[:nvocab, :], scalar2=None,
                                        op0=ALU.is_equal)
                pse = pp.tile([DIM, 512], f32, tag="ps")
                nc.tensor.matmul(pse[:, :cs], emb_t[:], oh[:, :cs], start=True, stop=True)
                nc.scalar.copy(e[:, l0:l0 + cs], pse[:, :cs])
            return e

        de = embed(d_idx[:], 65, LD, embd_t, "d")
        pe = embed(p_idx[:], 26, LP, embp_t, "p")

        # ---- conv stacks ----
        def conv(x, Lout, K, w_t, b_t, cout, tag, oc=None):
            """VALID conv via K accumulated matmuls. w_t: list of [Cin, CoutFull] tiles.
            oc: output-channel chunk (slice of 80) or None for full cout<=128."""
            y = ap_.tile([cout, Lout], f32, tag=tag)
            for l0 in range(0, Lout, 512):
                cs = min(512, Lout - l0)
                ps = pp.tile([cout, 512], f32, tag="ps")
                for k in range(K):
                    w = w_t[k][:] if oc is None else w_t[k][:, oc * 80:(oc + 1) * 80]
                    nc.tensor.matmul(ps[:, :cs], w, x[:, l0 + k:l0 + k + cs],
                                     start=(k == 0), stop=(k == K - 1))
                nc.scalar.activation(y[:, l0:l0 + cs], ps[:, :cs], AF.Relu, bias=b_t[:])
            return y

        dc1 = conv(de, LD1, 4, dw1_t, db1_t, CONV, "dc1")
        dc2 = conv(dc1, LD2, 6, dw2_t, db2_t, 2 * CONV, "dc2")
        dc = [conv(dc2, LD3, 8, dw3_t, db3_t[o], 80, f"dc3_{o}", oc=o) for o in range(2)]
        pc1 = conv(pe, LP1, 4, pw1_t, pb1_t, CONV, "pc1")
        pc2 = conv(pc1, LP2, 8, pw2_t, pb2_t, 2 * CONV, "pc2")
        pc = [conv(pc2, LP3, 12, pw3_t, pb3_t[o], 80, f"pc3_{o}", oc=o) for o in range(2)]

        # ---- attention projections: D[oc][80,85], P[oc][80,979] ----
        def att_proj(src, L, w_t, b_t, tag):
            res = []
            for oc in range(2):
                y = ap_.tile([80, L], f32, tag=f"{tag}_{oc}")
                for l0 in range(0, L, 512):
                    cs = min(512, L - l0)
                    ps = pp.tile([80, 512], f32, tag="ps")
                    for cc in range(2):
                        nc.tensor.matmul(ps[:, :cs], w_t[cc][:, oc * 80:(oc + 1) * 80],
                                         src[cc][:, l0:l0 + cs],
                                         start=(cc == 0), stop=(cc == 1))
                    nc.scalar.activation(y[:, l0:l0 + cs], ps[:, :cs], AF.Identity,
                                         bias=b_t[oc][:])
                res.append(y)
            return res

        D = att_proj(dc, LD3, daw_t, dab_t, "D")
        P = att_proj(pc, LP3, paw_t, pab_t, "P")

        # ---- R loop: S[cc][80,85] = sum_j relu ; T[cc][80,979] = sum_i relu ----
        S, T = [], []
        for cc in range(2):
            s = ap_.tile([80, LD3], f32, tag=f"S_{cc}")
            t0 = pT.tile([80, 512], f32, tag=f"T0_{cc}")
            t1 = pT.tile([80, LP3 - 512], f32, tag=f"T1_{cc}")
            for i in range(LD3):
                tm = tp.tile([80, LP3], f32, tag="rtmp")
                nc.scalar.activation(tm[:], P[cc][:], AF.Relu, bias=D[cc][:, i:i + 1],
                                     accum_out=s[:, i:i + 1])
                nc.tensor.matmul(t0[:], ident_t[:], tm[:, 0:512],
                                 start=(i == 0), stop=(i == LD3 - 1))
                nc.tensor.matmul(t1[:], ident_t[:], tm[:, 512:LP3],
                                 start=(i == 0), stop=(i == LD3 - 1))
            tsb = ap_.tile([80, LP3], f32, tag=f"T_{cc}")
            nc.scalar.copy(tsb[:, 0:512], t0[:])
            nc.scalar.copy(tsb[:, 512:LP3], t1[:])
            S.append(s)
            T.append(tsb)

        # ---- attention outputs: sigmoid((mean @ att_w + att_b)) ----
        ca, pa = [], []
        for oc in range(2):
            ps = pp.tile([80, 512], f32, tag="ps")
            for cc in range(2):
                nc.tensor.matmul(ps[:, :LD3], aw_t[cc][:, oc * 80:(oc + 1) * 80],
                                 S[cc][:], start=(cc == 0), stop=(cc == 1))
            y = ap_.tile([80, LD3], f32, tag=f"ca_{oc}")
            nc.scalar.activation(y[:], ps[:, :LD3], AF.Sigmoid, bias=ab_t[oc][:],
                                 scale=1.0 / LP3)
            ca.append(y)
        for oc in range(2):
            y = ap_.tile([80, LP3], f32, tag=f"pa_{oc}")
            for l0 in range(0, LP3, 512):
                cs = min(512, LP3 - l0)
                ps = pp.tile([80, 512], f32, tag="ps")
                for cc in range(2):
                    nc.tensor.matmul(ps[:, :cs], aw_t[cc][:, oc * 80:(oc + 1) * 80],
                                     T[cc][:, l0:l0 + cs], start=(cc == 0), stop=(cc == 1))
                nc.scalar.activation(y[:, l0:l0 + cs], ps[:, :cs], AF.Sigmoid,
                                     bias=ab_t[oc][:], scale=1.0 / LD3)
            pa.append(y)

        # ---- gate + global max pool ----
        vecs = []
        for (src, atte, L, tag) in [(dc, ca, LD3, "d"), (pc, pa, LP3, "p")]:
            for oc in range(2):
                g = tp.tile([80, L], f32, tag=f"g_{tag}")
                nc.vector.tensor_scalar(out=g[:], in0=atte[oc][:], scalar1=0.5,
                                        scalar2=None, op0=ALU.add)
                m = tp.tile([80, L], f32, tag=f"m_{tag}")
                nc.vector.tensor_tensor(out=m[:], in0=src[oc][:], in1=g[:], op=ALU.mult)
                v = ap_.tile([80, 1], f32, tag=f"v_{tag}{oc}")
                nc.vector.reduce_max(v[:], m[:], axis=AX.X)
                vecs.append(v)
        # vecs order: dvec0, dvec1, pvec0, pvec1 == pair[0:80],[80:160],[160:240],[240:320]

        # ---- FC head ----
        def lrelu_bias(ps, b_t, ncols, tag):
            h = ap_.tile([128, ncols], f32, tag=f"h_{tag}")
            nc.vector.tensor_tensor(out=h[:], in0=ps[:, :ncols], in1=b_t[:], op=ALU.add)
            t1 = tp.tile([128, ncols], f32, tag="fct")
            nc.vector.tensor_scalar(out=t1[:], in0=h[:], scalar1=0.01, scalar2=None,
                                    op0=ALU.mult)
            h2 = ap_.tile([128, ncols], f32, tag=f"h2_{tag}")
            nc.vector.tensor_tensor(out=h2[:], in0=h[:], in1=t1[:], op=ALU.max)
            return h2

        ps1 = pp.tile([128, 8], f32, tag="ps")
        for oc in range(8):
            for g in range(4):
                nc.tensor.matmul(ps1[:, oc:oc + 1], fc1w_t[g][:, oc * 128:(oc + 1) * 128],
                                 vecs[g][:], start=(g == 0), stop=(g == 3))
        h1 = lrelu_bias(ps1, fc1b_t, 8, "1")

        ps2 = pp.tile([128, 8], f32, tag="ps")
        for oc in range(8):
            for g in range(8):
                nc.tensor.matmul(ps2[:, oc:oc + 1], fc2w_t[g][:, oc * 128:(oc + 1) * 128],
                                 h1[:, g:g + 1], start=(g == 0), stop=(g == 7))
        h2 = lrelu_bias(ps2, fc2b_t, 8, "2")

        ps3 = pp.tile([128, 4], f32, tag="ps")
        for oc in range(4):
            for g in range(8):
                nc.tensor.matmul(ps3[:, oc:oc + 1], fc3w_t[g][:, oc * 128:(oc + 1) * 128],
                                 h2[:, g:g + 1], start=(g == 0), stop=(g == 7))
        h3 = lrelu_bias(ps3, fc3b_t, 4, "3")

        pso = pp.tile([2, 1], f32, tag="ps")
        for g in range(4):
            nc.tensor.matmul(pso[:], outw_t[g][:], h3[:, g:g + 1],
                             start=(g == 0), stop=(g == 3))
        ob = ap_.tile([2, 1], f32, tag="ob")
        nc.scalar.activation(ob[:], pso[:], AF.Identity, bias=outb_t[:])
        nc.sync.dma_start(out=out_d[:], in_=ob[:])

    nc.compile()
    return nc


def _prep_inputs(inputs):
    """Host-side layout prep. Returns (shared_params, per_core_fn)."""
    f = lambda x: np.ascontiguousarray(np.asarray(x), dtype=np.float32)
    shared = {
        "drug_emb": f(inputs["drug_emb"]),
        "prot_emb": f(inputs["prot_emb"]),
        "dw1t": f(np.transpose(inputs["dw1"], (2, 1, 0))),
        "dw2t": f(np.transpose(inputs["dw2"], (2, 1, 0))),
        "dw3t": f(np.transpose(inputs["dw3"], (2, 1, 0))),
        "pw1t": f(np.transpose(inputs["pw1"], (2, 1, 0))),
        "pw2t": f(np.transpose(inputs["pw2"], (2, 1, 0))),
        "pw3t": f(np.transpose(inputs["pw3"], (2, 1, 0))),
        "db1": f(inputs["db1"]).reshape(CONV, 1),
        "db2": f(inputs["db2"]).reshape(2 * CONV, 1),
        "db3": f(inputs["db3"]).reshape(2, 80, 1),
        "pb1": f(inputs["pb1"]).reshape(CONV, 1),
        "pb2": f(inputs["pb2"]).reshape(2 * CONV, 1),
        "pb3": f(inputs["pb3"]).reshape(2, 80, 1),
        "d_att_w": f(inputs["d_att_w"]),
        "p_att_w": f(inputs["p_att_w"]),
        "att_w": f(inputs["att_w"]),
        "d_att_b": f(inputs["d_att_b"]).reshape(2, 80, 1),
        "p_att_b": f(inputs["p_att_b"]).reshape(2, 80, 1),
        "att_b": f(inputs["att_b"]).reshape(2, 80, 1),
        "fc1_w": f(inputs["fc1_w"]),
        "fc1_b": f(inputs["fc1_b"]).reshape(8, 128).T.copy(),
        "fc2_w": f(inputs["fc2_w"]),
        "fc2_b": f(inputs["fc2_b"]).reshape(8, 128).T.copy(),
        "fc3_w": f(inputs["fc3_w"]),
        "fc3_b": f(inputs["fc3_b"]).reshape(4, 128).T.copy(),
        "out_w": f(inputs["out_w"]),
        "out_b": f(inputs["out_b"]).reshape(2, 1),
        "ones1": np.ones((1, 128), np.float32),
        "iota": np.arange(128, dtype=np.float32).reshape(128, 1),
        "ident80": np.eye(80, dtype=np.float32),
    }
    drug = np.asarray(inputs["drug"]).astype(np.float32)
    prot = np.asarray(inputs["protein"]).astype(np.float32)

    def per_core(i):
        m = dict(shared)
        m["drug_idx"] = np.ascontiguousarray(drug[i:i + 1])
        m["prot_idx"] = np.ascontiguousarray(prot[i:i + 1])
        return m

    return shared, per_core


def kernel(**inputs):
    from concourse.bass_utils import run_bass_kernel_spmd

    if "nc" not in _CACHE:
        _CACHE["nc"] = _build()
    nc = _CACHE["nc"]
    _, per_core = _prep_inputs(inputs)
    in_maps = [per_core(i) for i in range(B)]
    r = run_bass_kernel_spmd(nc, in_maps, core_ids=list(range(B)))
    out = np.stack([r.results[i]["out"].reshape(2) for i in range(B)])
    return out.astype(np.float32)


# revision 4
# speedup vs baseline: 4.9309x; 4.9309x over previous
"""AttentionDTI forward pass on 8 TRN2 NeuronCores — pure data parallel over batch.

Model (B=8, LD=100, LP=1000, DIM=64, CONV=40, C4=160):
  embed -> 3x conv1d+relu (drug: k=4,6,8 ; protein: k=4,8,12)
  d_att = dc^T @ d_att_w + b ; p_att = pc^T @ p_att_w + b
  R = relu(d_att[:,i,None,:] + p_att[:,None,j,:])      # [B,85,979,160] never materialized
  comp_atte = sigmoid((R.mean(2) @ att_w + att_b)^T)   # via S[c,i] = sum_j relu(...)
  prot_atte = sigmoid((R.mean(1) @ att_w + att_b)^T)   # via T[c,j] = sum_i relu(...)
  gate, global max pool, FC 320->1024->1024->512->2 (leaky relu 0.01)

Sharding: core b handles batch element b. All params replicated. No collectives.

The 160-wide channel dim is split A = 0:128 (full lanes) and B = 128:160 (32
lanes). For the R loop, chunk B is replicated x4 across lanes so 4 different
i-values are processed per instruction (85 iters -> 22). Per R iteration,
ScalarE or VectorE computes tmp = relu(P + D[:,i]) with fused free-axis
accumulation (S column); TensorE accumulates T += tmp via identity matmul into
PSUM (an [eye(32) x4] stack folds the 4 lane groups for chunk B).
All TensorE-facing tensors are bf16; accumulations stay f32.
"""

import numpy as np

B, LD, LP, DIM, CONV = 8, 100, 1000, 64, 40
C4 = 160
LD1, LD2, LD3 = 97, 92, 85     # drug conv output lengths (k=4,6,8)
LP1, LP2, LP3 = 997, 990, 979  # protein conv output lengths (k=4,8,12)
NB = 22                        # ceil(85/4) packed iterations for chunk B
ACT_FRAC = 0.47                # fraction of R iterations on ScalarE (rest on DVE)

_CACHE = {}


def _build():
    from contextlib import ExitStack
    import concourse.bass as bass
    import concourse.tile as tile
    from concourse import bacc, mybir

    f32 = mybir.dt.float32
    bf16 = mybir.dt.bfloat16
    AF = mybir.ActivationFunctionType
    ALU = mybir.AluOpType
    AX = mybir.AxisListType

    nc = bacc.Bacc("TRN2", target_bir_lowering=False, debug=False)

    def par(name, shape, dt=bf16):
        return nc.declare_dram_parameter(name, list(shape), dt, isOutput=False)

    # per-core inputs (f32-exact ints in bf16 range)
    d_idx = par("drug_idx", [1, LD])
    p_idx = par("prot_idx", [1, LP])
    # replicated params — bf16 unless accumulation/bias needs f32
    emb_d = par("drug_emb", [65, DIM])
    emb_p = par("prot_emb", [26, DIM])
    dw1t = par("dw1t", [4, DIM, CONV])
    dw2t = par("dw2t", [6, CONV, 2 * CONV])
    dw3t = par("dw3t", [8, 2 * CONV, C4])
    pw1t = par("pw1t", [4, DIM, CONV])
    pw2t = par("pw2t", [8, CONV, 2 * CONV])
    pw3t = par("pw3t", [12, 2 * CONV, C4])
    db1 = par("db1", [CONV, 1], f32)
    db2 = par("db2", [2 * CONV, 1], f32)
    db3 = par("db3", [C4, 1], f32)
    pb1 = par("pb1", [CONV, 1], f32)
    pb2 = par("pb2", [2 * CONV, 1], f32)
    pb3 = par("pb3", [C4, 1], f32)
    daw = par("d_att_w", [C4, C4])
    paw = par("p_att_w", [C4, C4])
    aw = par("att_w", [C4, C4])
    daw_r = par("daw_rep", [C4, 128])   # cols 128:160 tiled x4
    paw_r = par("paw_rep", [C4, 128])
    dab = par("d_att_b", [C4, 1], f32)
    pab = par("p_att_b", [C4, 1], f32)
    ab = par("att_b", [C4, 1], f32)
    dab_r = par("dab_rep", [128, 1], f32)  # b[128:160] tiled x4
    pab_r = par("pab_rep", [128, 1], f32)
    fc1w = par("fc1_w", [320, 1024])
    fc1b = par("fc1_b", [128, 8], f32)
    fc2w = par("fc2_w", [1024, 1024])
    fc2b = par("fc2_b", [128, 8], f32)
    fc3w = par("fc3_w", [1024, 512])
    fc3b = par("fc3_b", [128, 4], f32)
    outw = par("out_w", [512, 2])
    outb = par("out_b", [2, 1], f32)
    ones1 = par("ones1", [1, 128])
    iota = par("iota", [128, 1], f32)
    ident = par("ident128", [128, 128])
    ident4 = par("ident32x4", [128, 32])

    out_d = nc.declare_dram_parameter("out", [2, 1], f32, isOutput=True)

    CH = [(0, 128), (128, 32)]  # (offset, width) chunks of the 160 dim

    with tile.TileContext(nc) as tc, ExitStack() as ctx:
        wp = ctx.enter_context(tc.tile_pool(name="w", bufs=1))
        ap_ = ctx.enter_context(tc.tile_pool(name="a", bufs=1))
        tp = ctx.enter_context(tc.tile_pool(name="t", bufs=6))
        pp = ctx.enter_context(tc.tile_pool(name="p", bufs=2, space="PSUM"))
        pT = ctx.enter_context(tc.tile_pool(name="pT", bufs=1, space="PSUM"))

        def load(dram_ap, shape, tag, dt=bf16):
            t = wp.tile(list(shape), dt, tag=tag)
            nc.sync.dma_start(out=t[:], in_=dram_ap)
            return t

        # ---- load weights/constants ----
        ones_t = load(ones1[:], [1, 128], "ones")
        iota_t = load(iota[:], [128, 1], "iota", f32)
        id_t = load(ident[:], [128, 128], "ident")
        id4_t = load(ident4[:], [128, 32], "ident4")
        embd_t = load(emb_d[:], [65, DIM], "embd")
        embp_t = load(emb_p[:], [26, DIM], "embp")
        dw1_t = [load(dw1t[k], [DIM, CONV], f"dw1_{k}") for k in range(4)]
        dw2_t = [load(dw2t[k], [CONV, 2 * CONV], f"dw2_{k}") for k in range(6)]
        dw3_t = [load(dw3t[k], [2 * CONV, C4], f"dw3_{k}") for k in range(8)]
        pw1_t = [load(pw1t[k], [DIM, CONV], f"pw1_{k}") for k in range(4)]
        pw2_t = [load(pw2t[k], [CONV, 2 * CONV], f"pw2_{k}") for k in range(8)]
        pw3_t = [load(pw3t[k], [2 * CONV, C4], f"pw3_{k}") for k in range(12)]
        db1_t = load(db1[:], [CONV, 1], "db1", f32)
        db2_t = load(db2[:], [2 * CONV, 1], "db2", f32)
        db3_t = [load(db3[o:o + w], [w, 1], f"db3_{o}", f32) for o, w in CH]
        pb1_t = load(pb1[:], [CONV, 1], "pb1", f32)
        pb2_t = load(pb2[:], [2 * CONV, 1], "pb2", f32)
        pb3_t = [load(pb3[o:o + w], [w, 1], f"pb3_{o}", f32) for o, w in CH]
        daw_t = [load(daw[o:o + w, :], [w, C4], f"daw_{o}") for o, w in CH]
        paw_t = [load(paw[o:o + w, :], [w, C4], f"paw_{o}") for o, w in CH]
        aw_t = [load(aw[o:o + w, :], [w, C4], f"aw_{o}") for o, w in CH]
        dawr_t = [load(daw_r[o:o + w, :], [w, 128], f"dawr_{o}") for o, w in CH]
        pawr_t = [load(paw_r[o:o + w, :], [w, 128], f"pawr_{o}") for o, w in CH]
        dab_t = [load(dab[o:o + w], [w, 1], f"dab_{o}", f32) for o, w in CH]
        pab_t = [load(pab[o:o + w], [w, 1], f"pab_{o}", f32) for o, w in CH]
        ab_t = [load(ab[o:o + w], [w, 1], f"ab_{o}", f32) for o, w in CH]
        dabr_t = load(dab_r[:], [128, 1], "dabr", f32)
        pabr_t = load(pab_r[:], [128, 1], "pabr", f32)
        FCCH = [(0, 128), (128, 32), (160, 128), (288, 32)]
        fc1w_t = [load(fc1w[o:o + w, :], [w, 1024], f"fc1w_{o}") for o, w in FCCH]
        fc2w_t = [load(fc2w[g * 128:(g + 1) * 128, :], [128, 1024], f"fc2w_{g}") for g in range(8)]
        fc3w_t = [load(fc3w[g * 128:(g + 1) * 128, :], [128, 512], f"fc3w_{g}") for g in range(8)]
        outw_t = [load(outw[g * 128:(g + 1) * 128, :], [128, 2], f"outw_{g}") for g in range(4)]
        fc1b_t = load(fc1b[:], [128, 8], "fc1b", f32)
        fc2b_t = load(fc2b[:], [128, 8], "fc2b", f32)
        fc3b_t = load(fc3b[:], [128, 4], "fc3b", f32)
        outb_t = load(outb[:], [2, 1], "outb", f32)

        # ---- one-hot + embedding ----
        def embed(idx_dram, nvocab, L, emb_t, tag):
            idx_t = ap_.tile([1, L], bf16, tag=f"idx_{tag}")
            nc.sync.dma_start(out=idx_t[:], in_=idx_dram)
            e = ap_.tile([DIM, L], bf16, tag=f"e_{tag}")
            for l0 in range(0, L, 512):
                cs = min(512, L - l0)
                psb = pp.tile([nvocab, 512], f32, tag="ps")
                nc.tensor.matmul(psb[:, :cs], ones_t[:, :nvocab], idx_t[:, l0:l0 + cs],
                                 start=True, stop=True)
                oh = tp.tile([nvocab, 512], bf16, tag="oh")
                nc.vector.tensor_scalar(out=oh[:, :cs], in0=psb[:, :cs],
                                        scalar1=iota_t[:nvocab, :], scalar2=None,
                                        op0=ALU.is_equal)
                pse = pp.tile([DIM, 512], f32, tag="ps")
                nc.tensor.matmul(pse[:, :cs], emb_t[:], oh[:, :cs], start=True, stop=True)
                nc.scalar.copy(e[:, l0:l0 + cs], pse[:, :cs])
            return e

        de = embed(d_idx[:], 65, LD, embd_t, "d")
        pe = embed(p_idx[:], 26, LP, embp_t, "p")

        # ---- conv stacks (bf16 in/out, f32 psum) ----
        def conv(x, Lout, K, w_t, b_t, cout, tag, oc=None):
            y = ap_.tile([cout, Lout], bf16, tag=tag)
            for l0 in range(0, Lout, 512):
                cs = min(512, Lout - l0)
                ps = pp.tile([cout, 512], f32, tag="ps")
                for k in range(K):
                    w = w_t[k][:] if oc is None else w_t[k][:, oc[0]:oc[0] + oc[1]]
                    nc.tensor.matmul(ps[:, :cs], w, x[:, l0 + k:l0 + k + cs],
                                     start=(k == 0), stop=(k == K - 1))
                nc.scalar.activation(y[:, l0:l0 + cs], ps[:, :cs], AF.Relu, bias=b_t[:])
            return y

        dc1 = conv(de, LD1, 4, dw1_t, db1_t, CONV, "dc1")
        dc2 = conv(dc1, LD2, 6, dw2_t, db2_t, 2 * CONV, "dc2")
        dc = [conv(dc2, LD3, 8, dw3_t, db3_t[j], CH[j][1], f"dc3_{j}", oc=CH[j])
              for j in range(2)]
        pc1 = conv(pe, LP1, 4, pw1_t, pb1_t, CONV, "pc1")
        pc2 = conv(pc1, LP2, 8, pw2_t, pb2_t, 2 * CONV, "pc2")
        pc = [conv(pc2, LP3, 12, pw3_t, pb3_t[j], CH[j][1], f"pc3_{j}", oc=CH[j])
              for j in range(2)]

        # ---- attention projections ----
        # out tiles: X_A [128, L] (chans 0:128) and X_B4 [128, L] (chans 128:160 x4 rep)
        def att_proj(src, L, w_t, wr_t, b_t, br_t, tag, dt_a):
            res = []
            for which in range(2):  # 0 = A, 1 = B4(replicated)
                y = ap_.tile([128, L], dt_a if which == 0 or tag == "D" else bf16,
                             tag=f"{tag}{which}")
                for l0 in range(0, L, 512):
                    cs = min(512, L - l0)
                    ps = pp.tile([128, 512], f32, tag="ps")
                    for j in range(2):
                        w = w_t[j][:, 0:128] if which == 0 else wr_t[j][:]
                        nc.tensor.matmul(ps[:, :cs], w, src[j][:, l0:l0 + cs],
                                         start=(j == 0), stop=(j == 1))
                    bias = b_t[0][:] if which == 0 else br_t[:]
                    nc.scalar.activation(y[:, l0:l0 + cs], ps[:, :cs], AF.Identity,
                                         bias=bias)
                res.append(y)
            return res

        # D tiles f32 (used as per-partition scalars); P tiles bf16 (streamed)
        D_A, D_B4 = att_proj(dc, LD3, daw_t, dawr_t, dab_t, dabr_t, "D", f32)
        P_A, P_B4 = att_proj(pc, LP3, paw_t, pawr_t, pab_t, pabr_t, "P", bf16)

        # pack D_B4 [128, 85] -> D_Bp [128, 22]: lane (32g+c), col t = D[128+c, 4t+g]
        D_Bpad = ap_.tile([128, 88], f32, tag="D_Bpad")
        nc.vector.memset(D_Bpad[:], -1e4)
        nc.vector.tensor_copy(D_Bpad[:, 0:85], D_B4[:])
        D_Bp = ap_.tile([128, NB], f32, tag="D_Bp")
        for g in range(4):
            nc.vector.tensor_copy(D_Bp[g * 32:(g + 1) * 32, :],
                                  D_Bpad[g * 32:(g + 1) * 32, g:88:4])

        zeros_t = ap_.tile([128, LP3], bf16, tag="zeros")
        nc.vector.memset(zeros_t[:], 0.0)

        # ---- R loops ----
        def r_loop(P_t, D_cols, n_iter, s_tile, psl, psh, id_tile, idw):
            for i in range(n_iter):
                tm = tp.tile([128, LP3], bf16, tag="rtmp")
                if i % 2 == 0:
                    nc.scalar.activation(tm[:], P_t[:], AF.Relu,
                                         bias=D_cols[:, i:i + 1],
                                         accum_out=s_tile[:, i:i + 1])
                else:
                    nc.vector.scalar_tensor_tensor(
                        out=tm[:], in0=P_t[:], scalar=D_cols[:, i:i + 1],
                        in1=zeros_t[:], op0=ALU.add, op1=ALU.max,
                        accum_out=s_tile[:, i:i + 1])
                nc.tensor.matmul(psl[:], id_tile[:, :idw], tm[:, 0:512],
                                 start=(i == 0), stop=(i == n_iter - 1))
                nc.tensor.matmul(psh[:], id_tile[:, :idw], tm[:, 512:LP3],
                                 start=(i == 0), stop=(i == n_iter - 1))

        S_A = ap_.tile([128, LD3], f32, tag="S_A")
        TA0 = pT.tile([128, 512], f32, tag="TA0")
        TA1 = pT.tile([128, LP3 - 512], f32, tag="TA1")
        r_loop(P_A, D_A, LD3, S_A, TA0, TA1, id_t, 128)

        S_B4 = ap_.tile([128, NB], f32, tag="S_B4")
        TB0 = pT.tile([32, 512], f32, tag="TB0")
        TB1 = pT.tile([32, LP3 - 512], f32, tag="TB1")
        r_loop(P_B4, D_Bp, NB, S_B4, TB0, TB1, id4_t, 32)

        # S -> bf16 rhs tiles: S_Ab [128, 85]; unpack S_B4 -> S_Bb [32, 85]
        S_Ab = ap_.tile([128, LD3], bf16, tag="S_Ab")
        nc.vector.tensor_copy(S_Ab[:], S_A[:])
        S_Bb = ap_.tile([32, LD3], bf16, tag="S_Bb")
        for g in range(4):
            cnt = NB if g == 0 else NB - 1
            nc.vector.tensor_copy(S_Bb[:, g:g + 4 * (cnt - 1) + 1:4],
                                  S_B4[g * 32:(g + 1) * 32, 0:cnt])
        # T psum -> bf16 sbuf
        T_Ab = ap_.tile([128, LP3], bf16, tag="T_Ab")
        nc.vector.tensor_copy(T_Ab[:, 0:512], TA0[:])
        nc.vector.tensor_copy(T_Ab[:, 512:LP3], TA1[:])
        T_Bb = ap_.tile([32, LP3], bf16, tag="T_Bb")
        nc.vector.tensor_copy(T_Bb[:, 0:512], TB0[:])
        nc.vector.tensor_copy(T_Bb[:, 512:LP3], TB1[:])
        S_ch = [S_Ab, S_Bb]
        T_ch = [T_Ab, T_Bb]

        # ---- attention outputs: sigmoid((sum/n) @ att_w + att_b) ----
        def atte(rhs_ch, L, scale, tag):
            res = []
            for which, (o, w) in enumerate(CH):
                y = ap_.tile([w, L], bf16, tag=f"{tag}{which}")
                for l0 in range(0, L, 512):
                    cs = min(512, L - l0)
                    ps = pp.tile([w, 512], f32, tag="ps")
                    for j in range(2):
                        nc.tensor.matmul(ps[:, :cs], aw_t[j][:, o:o + w],
                                         rhs_ch[j][:, l0:l0 + cs],
                                         start=(j == 0), stop=(j == 1))
                    nc.scalar.activation(y[:, l0:l0 + cs], ps[:, :cs], AF.Sigmoid,
                                         bias=ab_t[which][:], scale=scale)
                res.append(y)
            return res

        ca = atte(S_ch, LD3, 1.0 / LP3, "ca")
        pa = atte(T_ch, LP3, 1.0 / LD3, "pa")

        # ---- gate + global max pool: v = max_l(src * (0.5 + atte)) ----
        vecs = {}
        for (src, att_, L, tag) in [(dc, ca, LD3, "d"), (pc, pa, LP3, "p")]:
            for which, (o, w) in enumerate(CH):
                g = tp.tile([w, L], bf16, tag=f"g_{tag}{which}")
                nc.vector.tensor_scalar(out=g[:], in0=att_[which][:], scalar1=0.5,
                                        scalar2=None, op0=ALU.add)
                m = tp.tile([w, L], bf16, tag=f"m_{tag}{which}")
                nc.vector.tensor_tensor(out=m[:], in0=src[which][:], in1=g[:],
                                        op=ALU.mult)
                v = ap_.tile([w, 1], bf16, tag=f"v_{tag}{which}")
                nc.vector.reduce_max(v[:], m[:], axis=AX.X)
                vecs[f"{tag}{which}"] = v
        # pair layout: [dvecA(128), dvecB(32), pvecA(128), pvecB(32)]
        vlist = [vecs["d0"], vecs["d1"], vecs["p0"], vecs["p1"]]

        # ---- FC head ----
        def lrelu_bias(ps, b_t, ncols, tag):
            h = ap_.tile([128, ncols], f32, tag=f"h_{tag}")
            nc.vector.tensor_tensor(out=h[:], in0=ps[:, :ncols], in1=b_t[:], op=ALU.add)
            t1 = tp.tile([128, ncols], f32, tag="fct")
            nc.vector.tensor_scalar(out=t1[:], in0=h[:], scalar1=0.01, scalar2=None,
                                    op0=ALU.mult)
            h2 = ap_.tile([128, ncols], bf16, tag=f"h2_{tag}")
            nc.vector.tensor_tensor(out=h2[:], in0=h[:], in1=t1[:], op=ALU.max)
            return h2

        ps1 = pp.tile([128, 8], f32, tag="ps")
        for oc in range(8):
            for g in range(4):
                nc.tensor.matmul(ps1[:, oc:oc + 1], fc1w_t[g][:, oc * 128:(oc + 1) * 128],
                                 vlist[g][:], start=(g == 0), stop=(g == 3))
        h1 = lrelu_bias(ps1, fc1b_t, 8, "1")

        ps2 = pp.tile([128, 8], f32, tag="ps")
        for oc in range(8):
            for g in range(8):
                nc.tensor.matmul(ps2[:, oc:oc + 1], fc2w_t[g][:, oc * 128:(oc + 1) * 128],
                                 h1[:, g:g + 1], start=(g == 0), stop=(g == 7))
        h2 = lrelu_bias(ps2, fc2b_t, 8, "2")

        ps3 = pp.tile([128, 4], f32, tag="ps")
        for oc in range(4):
            for g in range(8):
                nc.tensor.matmul(ps3[:, oc:oc + 1], fc3w_t[g][:, oc * 128:(oc + 1) * 128],
                                 h2[:, g:g + 1], start=(g == 0), stop=(g == 7))
        h3 = lrelu_bias(ps3, fc3b_t, 4, "3")

        pso = pp.tile([2, 1], f32, tag="ps")
        for g in range(4):
            nc.tensor.matmul(pso[:], outw_t[g][:], h3[:, g:g + 1],
                             start=(g == 0), stop=(g == 3))
        ob = ap_.tile([2, 1], f32, tag="ob")
        nc.scalar.activation(ob[:], pso[:], AF.Identity, bias=outb_t[:])
        nc.sync.dma_start(out=out_d[:], in_=ob[:])

    nc.compile()
    return nc


def _prep_inputs(inputs):
    """Host-side layout prep. Returns (shared_params, per_core_fn)."""
    import ml_dtypes
    bf = lambda x: np.ascontiguousarray(np.asarray(x), dtype=ml_dtypes.bfloat16)
    f = lambda x: np.ascontiguousarray(np.asarray(x), dtype=np.float32)
    rep4 = lambda x: np.tile(x, (4,) + (1,) * (x.ndim - 1))  # tile x4 along axis0
    shared = {
        "drug_emb": bf(inputs["drug_emb"]),
        "prot_emb": bf(inputs["prot_emb"]),
        "dw1t": bf(np.transpose(inputs["dw1"], (2, 1, 0))),
        "dw2t": bf(np.transpose(inputs["dw2"], (2, 1, 0))),
        "dw3t": bf(np.transpose(inputs["dw3"], (2, 1, 0))),
        "pw1t": bf(np.transpose(inputs["pw1"], (2, 1, 0))),
        "pw2t": bf(np.transpose(inputs["pw2"], (2, 1, 0))),
        "pw3t": bf(np.transpose(inputs["pw3"], (2, 1, 0))),
        "db1": f(inputs["db1"]).reshape(CONV, 1),
        "db2": f(inputs["db2"]).reshape(2 * CONV, 1),
        "db3": f(inputs["db3"]).reshape(C4, 1),
        "pb1": f(inputs["pb1"]).reshape(CONV, 1),
        "pb2": f(inputs["pb2"]).reshape(2 * CONV, 1),
        "pb3": f(inputs["pb3"]).reshape(C4, 1),
        "d_att_w": bf(inputs["d_att_w"]),
        "p_att_w": bf(inputs["p_att_w"]),
        "att_w": bf(inputs["att_w"]),
        "daw_rep": bf(np.tile(np.asarray(inputs["d_att_w"])[:, 128:160], (1, 4))),
        "paw_rep": bf(np.tile(np.asarray(inputs["p_att_w"])[:, 128:160], (1, 4))),
        "d_att_b": f(inputs["d_att_b"]).reshape(C4, 1),
        "p_att_b": f(inputs["p_att_b"]).reshape(C4, 1),
        "att_b": f(inputs["att_b"]).reshape(C4, 1),
        "dab_rep": f(rep4(np.asarray(inputs["d_att_b"])[128:160])).reshape(128, 1),
        "pab_rep": f(rep4(np.asarray(inputs["p_att_b"])[128:160])).reshape(128, 1),
        "fc1_w": bf(inputs["fc1_w"]),
        "fc1_b": f(inputs["fc1_b"]).reshape(8, 128).T.copy(),
        "fc2_w": bf(inputs["fc2_w"]),
        "fc2_b": f(inputs["fc2_b"]).reshape(8, 128).T.copy(),
        "fc3_w": bf(inputs["fc3_w"]),
        "fc3_b": f(inputs["fc3_b"]).reshape(4, 128).T.copy(),
        "out_w": bf(inputs["out_w"]),
        "out_b": f(inputs["out_b"]).reshape(2, 1),
        "ones1": np.ones((1, 128), ml_dtypes.bfloat16),
        "iota": np.arange(128, dtype=np.float32).reshape(128, 1),
        "ident128": np.eye(128, dtype=ml_dtypes.bfloat16),
        "ident32x4": np.tile(np.eye(32, dtype=ml_dtypes.bfloat16), (4, 1)),
    }
    drug = np.asarray(inputs["drug"]).astype(ml_dtypes.bfloat16)
    prot = np.asarray(inputs["protein"]).astype(ml_dtypes.bfloat16)

    def per_core(i):
        m = dict(shared)
        m["drug_idx"] = np.ascontiguousarray(drug[i:i + 1])
        m["prot_idx"] = np.ascontiguousarray(prot[i:i + 1])
        return m

    return shared, per_core


def kernel(**inputs):
    from concourse.bass_utils import run_bass_kernel_spmd

    if "nc" not in _CACHE:
        _CACHE["nc"] = _build()
    nc = _CACHE["nc"]
    _, per_core = _prep_inputs(inputs)
    in_maps = [per_core(i) for i in range(B)]
    r = run_bass_kernel_spmd(nc, in_maps, core_ids=list(range(B)))
    out = np.stack([r.results[i]["out"].reshape(2) for i in range(B)])
    return out.astype(np.float32)
